# revision 1
# baseline (speedup 1.0000x reference)
"""Cross-frame attention kernel for 8 TRN2 NeuronCores.

Sharding: core c handles batch b = c//2 and head-group g = c%2 (4 of the 8
heads).  The host pre-transposes x[b]/context[b] (feature dim onto SBUF
partitions) and casts to bf16; each core computes a partial output
(its 4 heads pushed through the matching Wo rows) and the host sums the
two partials per batch plus the bias.

Device math per core (S^T layout, softmax over the partition j-dim):
  QT = Wq_g^T x^T          [256, 2048]
  KT = Wk_g^T c^T          [256, 2048]
  V  = c Wv_g              [2048, 256] (+ ones column per head)
  S^T = K_h Q_h^T          [j, i] tiles, exp via ScalarE (scale=1/8 fused)
  O~^T | Z = [V_h|1]^T expS^T   (PSUM accumulate over j)
  A^T = O~^T * bcast(1/Z)  (K=1 broadcast matmul for the free-dim scale)
  out_partial = A^T^T Wo_g [2048, 512] fp32

Logits are |S/8| <~ 1.1 for this problem's scale, so softmax without
max-subtraction is exact in fp32.
"""

import numpy as np
import ml_dtypes

B = 4
N = 2048  # query length
M = 2048  # context length
DIM = 512
HEADS = 8
DH = 64
HC = 256  # head columns handled per core (4 heads)
P = 128
KO = DIM // P  # 4 k-chunks
NI4 = N // 512  # 4 i-chunks of 512
NJ = M // P  # 16 j-chunks
JPG = 2  # j-chunks per exp group (PSUM banks per S^T buffer)

_CACHE = {}


def _build():
    from contextlib import ExitStack

    import concourse.mybir as mybir
    import concourse.tile as tile
    from concourse import bacc

    bf = mybir.dt.bfloat16
    f32 = mybir.dt.float32
    Exp = mybir.ActivationFunctionType.Exp

    nc = bacc.Bacc(None, target_bir_lowering=False, debug=False)
    with tile.TileContext(nc) as tc:
        with ExitStack() as ctx:
            dram = ctx.enter_context(tc.tile_pool(name="dram", bufs=1, space="DRAM"))
            xT_d = dram.tile([DIM, N], bf, kind="ExternalInput")
            cT_d = dram.tile([DIM, M], bf, kind="ExternalInput")
            wq_d = dram.tile([DIM, HC], bf, kind="ExternalInput")
            wk_d = dram.tile([DIM, HC], bf, kind="ExternalInput")
            wv_d = dram.tile([DIM, HC], bf, kind="ExternalInput")
            wo_d = dram.tile([HC, DIM], bf, kind="ExternalInput")
            out_d = dram.tile([N, DIM], f32, kind="ExternalOutput")

            const = ctx.enter_context(tc.tile_pool(name="const", bufs=1))

            xt_sb = const.tile([P, KO, N], bf, tag="xt")
            ct_sb = const.tile([P, KO, M], bf, tag="ct")
            wq_sb = const.tile([P, KO, HC], bf, tag="wq")
            wk_sb = const.tile([P, KO, HC], bf, tag="wk")
            wv_sb = const.tile([P, KO, HC], bf, tag="wv")
            wo_sb = const.tile([P, 2, DIM], bf, tag="wo")
            qT_sb = const.tile([P, 2, N], bf, tag="qT")
            kT_sb = const.tile([P, 2, M], bf, tag="kT")
            # all 4 heads' V with a trailing ones column: [j, jo, head, 65]
            vp_sb = const.tile([P, NJ, 4, DH + 1], bf, tag="vp")
            aT_sb = const.tile([P, 2, N], bf, tag="aT")
            ones_sb = const.tile([1, DH], bf, tag="ones1")

            dummy_sb = const.tile([1, 1], f32, tag="dummy")
            nc.vector.memset(ones_sb[:], 1.0)
            nc.vector.memset(vp_sb[:, :, :, DH : DH + 1], 1.0)
            # hoist the exp ACT-table load out of the critical path
            nc.scalar.activation(dummy_sb[:], ones_sb[0:1, 0:1], Exp, scale=1.0)

            # DMA in (ko, i4) pieces, first-needed first: kT/qT chunk-0
            # inputs, then V weights, then the rest in i4 order.
            nc.sync.dma_start(wk_sb[:], wk_d[:].rearrange("(ko p) m -> p ko m", p=P))
            nc.sync.dma_start(wq_sb[:], wq_d[:].rearrange("(ko p) m -> p ko m", p=P))
            cT_r = cT_d[:].rearrange("(ko p) i -> p ko i", p=P)
            xT_r = xT_d[:].rearrange("(ko p) i -> p ko i", p=P)
            for ko in range(KO):
                nc.sync.dma_start(ct_sb[:, ko, 0:512], cT_r[:, ko, 0:512])
            for ko in range(KO):
                nc.sync.dma_start(xt_sb[:, ko, 0:512], xT_r[:, ko, 0:512])
            nc.sync.dma_start(wv_sb[:], wv_d[:].rearrange("(ko p) m -> p ko m", p=P))
            for i4 in range(1, NI4):
                isl = slice(i4 * 512, (i4 + 1) * 512)
                for ko in range(KO):
                    nc.sync.dma_start(ct_sb[:, ko, isl], cT_r[:, ko, isl])
                for ko in range(KO):
                    nc.sync.dma_start(xt_sb[:, ko, isl], xT_r[:, ko, isl])
            nc.sync.dma_start(wo_sb[:], wo_d[:].rearrange("(r p) n -> p r n", p=P))

            # Single shared PSUM budget (8 banks):
            #   s-tag 2x2 + ppv 2x1 + o 1 + scr 1
            with (
                tc.tile_pool(name="s_ps", bufs=2, space="PSUM") as s_pool,
                tc.tile_pool(name="aux_ps", bufs=1, space="PSUM") as aux_pool,
                tc.tile_pool(name="o_ps", bufs=2, space="PSUM") as o_pool,
                tc.tile_pool(name="scr_ps", bufs=1, space="PSUM") as scr_pool,
                tc.tile_pool(name="e_sb", bufs=4) as e_pool,
                tc.tile_pool(name="small", bufs=2) as small,
                tc.tile_pool(name="ost", bufs=2) as ostp,
            ):
                def qk_proj(wsb, src_sb, dst, m, chunks):
                    for i4 in chunks:
                        ps = aux_pool.tile([P, 512], f32, tag="aux", name="ps_qk")
                        for ko in range(KO):
                            nc.tensor.matmul(
                                ps[:],
                                wsb[:, ko, m * P : (m + 1) * P],
                                src_sb[:, ko, i4 * 512 : (i4 + 1) * 512],
                                start=(ko == 0),
                                stop=(ko == KO - 1),
                            )
                        nc.vector.tensor_copy(
                            dst[:, m, i4 * 512 : (i4 + 1) * 512], ps[:]
                        )

                def att_block(i4, m, hl, fillers=None, vfill=False):
                    isl = slice(i4 * 512, (i4 + 1) * 512)
                    h = 2 * m + hl
                    pb = DH * hl
                    o_ps = o_pool.tile([DH + 1, 512], f32, tag="o", name="o_ps")
                    for jg in range(NJ // JPG):
                        for f in (fillers or {}).get(jg, []):
                            f()
                        if vfill:
                            vpair(jg)()
                        s_ps = s_pool.tile([P, JPG, 512], f32, tag="s", name="s_ps")
                        for jj in range(JPG):
                            j = jg * JPG + jj
                            nc.tensor.matmul(
                                s_ps[:, jj, :],
                                kT_sb[pb : pb + DH, m, j * P : (j + 1) * P],
                                qT_sb[pb : pb + DH, m, isl],
                                start=True,
                                stop=True,
                            )
                        e_sb = e_pool.tile([P, JPG, 512], bf, tag="e", name="e_sb")
                        nc.scalar.activation(e_sb[:], s_ps[:], Exp, scale=0.125)
                        for jj in range(JPG):
                            j = jg * JPG + jj
                            nc.tensor.matmul(
                                o_ps[:],
                                vp_sb[:, j, h, :],
                                e_sb[:, jj, :],
                                start=(j == 0),
                                stop=(j == NJ - 1),
                            )
                    rz = small.tile([1, 512], f32, tag="rz", name="rz")
                    nc.vector.reciprocal(rz[:], o_ps[DH : DH + 1, :])
                    rzb = small.tile([1, 512], bf, tag="rzb", name="rzb")
                    nc.vector.tensor_copy(rzb[:], rz[:])
                    bc = scr_pool.tile([DH, 512], f32, tag="scr", name="bc")
                    nc.tensor.matmul(bc[:], ones_sb[:], rzb[:], start=True, stop=True)
                    bcb = small.tile([DH, 512], bf, tag="bcb", name="bcb")
                    nc.vector.tensor_copy(bcb[:], bc[:])
                    nc.vector.tensor_mul(
                        aT_sb[pb : pb + DH, m, isl], o_ps[0:DH, :], bcb[:]
                    )

                def wo_proj(i4):
                    for ii in range(4):
                        i = i4 * 4 + ii
                        ps = scr_pool.tile([P, DIM], f32, tag="scr", name="p3_ps")
                        for m in range(2):
                            nc.tensor.matmul(
                                ps[:],
                                aT_sb[:, m, i * P : (i + 1) * P],
                                wo_sb[:, m, :],
                                start=(m == 0),
                                stop=(m == 1),
                            )
                        ost = ostp.tile([P, DIM], f32, tag="ost", name="ost")
                        nc.vector.tensor_copy(ost[:], ps[:])
                        nc.sync.dma_start(out_d[i * P : (i + 1) * P, :], ost[:])

                def kchunk(m, c):
                    return lambda: qk_proj(wk_sb, ct_sb, kT_sb, m, [c])

                def qchunk(m, c):
                    return lambda: qk_proj(wq_sb, xt_sb, qT_sb, m, [c])

                def vpair(g):
                    def f():
                        for jo in (2 * g, 2 * g + 1):
                            ps = aux_pool.tile([P, HC], f32, tag="aux", name="ps_v")
                            for ko in range(KO):
                                nc.tensor.matmul(
                                    ps[:],
                                    ct_sb[:, ko, jo * P : (jo + 1) * P],
                                    wv_sb[:, ko, :],
                                    start=(ko == 0),
                                    stop=(ko == KO - 1),
                                )
                            nc.vector.tensor_copy(
                                vp_sb[:, jo, :, 0:DH],
                                ps[:].rearrange("p (h d) -> p h d", h=4),
                            )
                    return f

                # m=0 blocks run one i4 ahead of m=1; projections drip in as
                # per-group fillers so they never gate the exp stream.
                qk_proj(wk_sb, ct_sb, kT_sb, 0, [0])
                qk_proj(wq_sb, xt_sb, qT_sb, 0, [0])
                att_block(0, 0, 0, {
                    1: [kchunk(0, 1)], 3: [kchunk(0, 2)], 5: [kchunk(0, 3)],
                    **{g: [] for g in ()},
                }, vfill=True)
                att_block(0, 0, 1, {
                    0: [qchunk(0, 1)], 2: [kchunk(1, 0)],
                    4: [kchunk(1, 1)], 6: [kchunk(1, 2)],
                })
                att_block(1, 0, 0, {
                    0: [kchunk(1, 3)], 2: [qchunk(1, 0)], 4: [qchunk(0, 2)],
                })
                att_block(1, 0, 1, {0: [qchunk(1, 1)]})
                att_block(0, 1, 0)
                att_block(0, 1, 1)
                wo_proj(0)
                att_block(2, 0, 0, {0: [qchunk(0, 3)], 4: [qchunk(1, 2)]})
                att_block(2, 0, 1)
                att_block(1, 1, 0)
                att_block(1, 1, 1)
                wo_proj(1)
                att_block(3, 0, 0, {0: [qchunk(1, 3)]})
                att_block(3, 0, 1)
                att_block(2, 1, 0)
                att_block(2, 1, 1)
                wo_proj(2)
                att_block(3, 1, 0)
                att_block(3, 1, 1)
                wo_proj(3)

    nc.compile()
    names = dict(
        xT=xT_d.name,
        cT=cT_d.name,
        wq=wq_d.name,
        wk=wk_d.name,
        wv=wv_d.name,
        wo=wo_d.name,
        out=out_d.name,
    )
    return nc, names


def _get_built():
    if "nc" not in _CACHE:
        _CACHE["nc"], _CACHE["names"] = _build()
    return _CACHE["nc"], _CACHE["names"]


def run(x, context, Wq, Wk, Wv, Wo, bo, trace=False):
    from concourse.bass_utils import run_bass_kernel_spmd

    nc, names = _get_built()
    bf16 = ml_dtypes.bfloat16

    x = np.asarray(x, dtype=np.float32)
    context = np.asarray(context, dtype=np.float32)
    Wq = np.asarray(Wq, dtype=np.float32)
    Wk = np.asarray(Wk, dtype=np.float32)
    Wv = np.asarray(Wv, dtype=np.float32)
    Wo = np.asarray(Wo, dtype=np.float32)
    bo = np.asarray(bo, dtype=np.float32)

    in_maps = []
    for c in range(8):
        b, g = divmod(c, 2)
        cols = slice(g * HC, (g + 1) * HC)
        in_maps.append(
            {
                names["xT"]: np.ascontiguousarray(x[b].T).astype(bf16),
                names["cT"]: np.ascontiguousarray(context[b].T).astype(bf16),
                names["wq"]: np.ascontiguousarray(Wq[:, cols]).astype(bf16),
                names["wk"]: np.ascontiguousarray(Wk[:, cols]).astype(bf16),
                names["wv"]: np.ascontiguousarray(Wv[:, cols]).astype(bf16),
                names["wo"]: np.ascontiguousarray(Wo[cols, :]).astype(bf16),
            }
        )

    res = run_bass_kernel_spmd(
        nc, in_maps, core_ids=list(range(8)), trace=trace,
        stitch_traces=trace,
    )
    out = np.empty((B, N, DIM), dtype=np.float32)
    for b in range(B):
        out[b] = res.results[2 * b][names["out"]] + res.results[2 * b + 1][names["out"]]
    out += bo[None, None, :]
    return out, res


def kernel(x, context, Wq, Wk, Wv, Wo, bo):
    out, _ = run(x, context, Wq, Wk, Wv, Wo, bo, trace=False)
    return out

